# revision 33
# baseline (speedup 1.0000x reference)
"""Causal multi-head self-attention on 8 TRN2 NeuronCores.

Sharding: core c handles batch b = c//2 and head-half hh = c%2 (8 of 16
heads).  Each core computes qkv projection for its heads, RoPE, causal
attention, and a PARTIAL output projection (its heads' contribution to
Wout @ attn).  The host sums the two half-head partials per batch.
No collectives.

Attention uses TRANSPOSED scores sT[k, q] (no on-chip transposes); two
heads are row-packed into the PE array per score matmul; exp runs as one
ACT op per chunk-pair; attn@V uses a ones-column for the softmax sums
(head A: lhsT=[v|1] M=65, sums on partition 64; head B:
lhsT=[0(32)|1|0(31)|v] M=128, sums on partition 32, output on partitions
64-127 -- keeps every vector op lane-aligned).  Normalization: broadcast
the sums row with a K=1 ones-matmul, fast approximate reciprocal on the
broadcast, one tensor_mul per head fused with the psum->sbuf cast.

The qkv projection + RoPE is emitted JUST-IN-TIME, woven into the
attention pair boundaries, so the (ACT-exp-bound) attention phase hides
the projection's PE work.  Projection psum cycles through the attn-
accumulator pool's slots serially (ps half -> release -> swap psum).
"""

import numpy as np
import ml_dtypes

BF16 = ml_dtypes.bfloat16

B, S, D = 4, 2048, 1024
H, DK = 16, 64
THETA = 10000.0
NCORES = 8
HLOC = H // 2
NPAIR = HLOC // 2
P = 128
SBLK = 512
OV = HLOC * DK
WCOLS = 3 * OV


def _rope_perm():
    return np.concatenate([np.arange(0, DK, 2), np.arange(1, DK, 2)])


def _host_tables(s):
    half = DK // 2
    inv_freq = THETA ** (-np.arange(0, DK, 2, dtype=np.float64) / DK)
    pos = np.arange(s, dtype=np.float64)
    ang = pos[None, :] * inv_freq[:, None]
    c, sn = np.cos(ang), np.sin(ang)
    cos_t = np.empty((P, s), np.float32)
    sin_t = np.empty((P, s), np.float32)
    for hrow in range(2):
        o = hrow * DK
        cos_t[o:o + half] = c
        cos_t[o + half:o + DK] = c
        sin_t[o:o + half] = -sn
        sin_t[o + half:o + DK] = sn
    scale = 1.0 / np.sqrt(DK)
    cosq = (cos_t * scale).astype(BF16)
    sinq = (sin_t * scale).astype(BF16)
    cosk = cos_t.astype(BF16)
    sink = sin_t.astype(BF16)

    swap = np.zeros((P, P), np.float32)
    for hrow in range(2):
        o = hrow * DK
        for i in range(half):
            swap[o + i, o + half + i] = 1.0
            swap[o + half + i, o + i] = 1.0
    swapm = swap.astype(BF16)

    tri = (np.arange(P)[:, None] <= np.arange(P)[None, :]).astype(np.float32)
    tri2 = np.concatenate([tri, tri], axis=1).astype(BF16)
    return cosq, sinq, cosk, sink, swapm, tri2


def _build_nc(s=S):
    import concourse.bass as bass  # noqa: F401
    import concourse.mybir as mybir
    import concourse.tile as tile
    from concourse import bacc
    from contextlib import ExitStack

    f32 = mybir.dt.float32
    bf16 = mybir.dt.bfloat16
    EXP = mybir.ActivationFunctionType.Exp
    MUL = mybir.AluOpType.mult

    nsb = s // SBLK
    nqc = s // P
    dch = D // P
    assert nsb % 2 == 0

    nc = bacc.Bacc(None, target_bir_lowering=False)
    xT_d = nc.dram_tensor("xT", [D, s], bf16, kind="ExternalInput")
    wq_d = nc.dram_tensor("wqkvT", [D, WCOLS], bf16, kind="ExternalInput")
    wo_d = nc.dram_tensor("woutT", [OV, D], bf16, kind="ExternalInput")
    cosq_d = nc.dram_tensor("cosq", [P, s], bf16, kind="ExternalInput")
    sinq_d = nc.dram_tensor("sinq", [P, s], bf16, kind="ExternalInput")
    cosk_d = nc.dram_tensor("cosk", [P, s], bf16, kind="ExternalInput")
    sink_d = nc.dram_tensor("sink", [P, s], bf16, kind="ExternalInput")
    swap_d = nc.dram_tensor("swapm", [P, P], bf16, kind="ExternalInput")
    tri_d = nc.dram_tensor("tri2", [P, 2 * P], bf16, kind="ExternalInput")
    out_d = nc.dram_tensor("out", [s, D], f32, kind="ExternalOutput")

    W2 = 2 * SBLK

    with tile.TileContext(nc) as tc, ExitStack() as ctx:
        const = ctx.enter_context(tc.tile_pool(name="const", bufs=1))
        psS = ctx.enter_context(
            tc.tile_pool(name="psS", bufs=2, space="PSUM"))
        psB = ctx.enter_context(
            tc.tile_pool(name="psB", bufs=4, space="PSUM"))
        rpool = ctx.enter_context(tc.tile_pool(name="rope", bufs=3))
        ppool = ctx.enter_context(tc.tile_pool(name="probs", bufs=6))
        npool = ctx.enter_context(tc.tile_pool(name="norm", bufs=3))
        opool = ctx.enter_context(tc.tile_pool(name="outsb", bufs=2))
        atpool = ctx.enter_context(tc.tile_pool(name="attnT", bufs=3))

        # ---- staged constant loads -----------------------------------------
        # order matters: v weights + first x column block land first so the
        # first v-projection units can start ~immediately.
        NCB = s // SBLK                     # x column blocks
        xT = [[None] * NCB for _ in range(dch)]
        wqv = []
        wqk = []
        for d in range(dch):
            t = const.tile([P, OV], bf16, tag=f"wqv{d}")
            nc.sync.dma_start(out=t, in_=wq_d[d * P:(d + 1) * P, 2 * OV:])
            wqv.append(t)
        for d in range(dch):
            t = const.tile([P, SBLK], bf16, tag=f"xT{d}_0")
            nc.sync.dma_start(out=t, in_=xT_d[d * P:(d + 1) * P, 0:SBLK])
            xT[d][0] = t
        for d in range(dch):
            t = const.tile([P, 2 * OV], bf16, tag=f"wqk{d}")
            nc.sync.dma_start(out=t, in_=wq_d[d * P:(d + 1) * P, 0:2 * OV])
            wqk.append(t)
        tabs = {}
        for nm, dram in (("cosq", cosq_d), ("sinq", sinq_d),
                         ("cosk", cosk_d), ("sink", sink_d)):
            t = const.tile([P, s], bf16, tag=nm)
            nc.sync.dma_start(out=t, in_=dram[:, :])
            tabs[nm] = t
        swap_sb = const.tile([P, P], bf16, tag="swapm")
        nc.sync.dma_start(out=swap_sb, in_=swap_d[:, :])
        tri_sb = const.tile([P, 2 * P], bf16, tag="tri2")
        nc.sync.dma_start(out=tri_sb, in_=tri_d[:, :])
        tri3 = tri_sb.rearrange("p (h q) -> p h q", h=2)
        ones_sb = const.tile([P, DK], bf16, tag="ones")
        nc.vector.memset(ones_sb, 1.0)
        # HAM warm-up: keep the PE busy on scratch data while the input DMAs
        # stream in, so the real matmuls start at the full 2.4 GHz clock.
        warm = const.tile([P, SBLK], bf16, tag="warm")
        nc.vector.memset(warm, 1.0)
        wps = psB.tile([P, SBLK], f32, tag="acc")
        for _ in range(24):
            nc.tensor.matmul(wps, warm[:, 0:P], warm,
                             start=True, stop=True)
        for cb in range(1, NCB):
            for d in range(dch):
                t = const.tile([P, SBLK], bf16, tag=f"xT{d}_{cb}")
                nc.sync.dma_start(
                    out=t, in_=xT_d[d * P:(d + 1) * P,
                                    cb * SBLK:(cb + 1) * SBLK])
                xT[d][cb] = t
        wo = []
        for i in range(OV // P):
            t = const.tile([P, D], bf16, tag=f"wo{i}")
            nc.sync.dma_start(out=t, in_=wo_d[i * P:(i + 1) * P, :])
            wo.append(t)

        # ---- projection units (emitted JIT) --------------------------------
        qt = [[None] * (nsb // 2) for _ in range(NPAIR)]   # [128, 1024]
        kt = [[None] * (nsb // 2) for _ in range(NPAIR)]
        vA = [[None] * NPAIR for _ in range(nqc)]
        vB = [[None] * NPAIR for _ in range(nqc)]
        done_qk = set()
        done_v = set()

        def emit_qk_unit(is_q, pr, sbp, half):
            key = (is_q, pr, sbp, half)
            if key in done_qk:
                return
            done_qk.add(key)
            dests = qt if is_q else kt
            if dests[pr][sbp] is None:
                nm = ("qt" if is_q else "kt") + f"{pr}_{sbp}"
                dests[pr][sbp] = const.tile(
                    [P, W2], bf16, tag=nm, name=nm)
            dest = dests[pr][sbp]
            wcol = (pr if is_q else NPAIR + pr) * P
            ct = tabs["cosq"] if is_q else tabs["cosk"]
            st = tabs["sinq"] if is_q else tabs["sink"]
            cb = 2 * sbp + half
            ps = psB.tile([P, SBLK], f32, tag="acc")
            for d in range(dch):
                nc.tensor.matmul(
                    ps, wqk[d][:, wcol:wcol + P], xT[d][cb],
                    start=(d == 0), stop=(d == dch - 1))
            c0 = cb * SBLK
            y = rpool.tile([P, SBLK], bf16, tag="y")
            nc.scalar.copy(y, ps)
            sw = psB.tile([P, SBLK], f32, tag="acc")
            nc.tensor.matmul(sw, swap_sb, y, start=True, stop=True)
            t1 = rpool.tile([P, SBLK], bf16, tag="t1")
            nc.vector.tensor_mul(t1, y, ct[:, c0:c0 + SBLK])
            t2 = rpool.tile([P, SBLK], bf16, tag="t2")
            nc.vector.tensor_mul(t2, sw, st[:, c0:c0 + SBLK])
            sl = slice(half * SBLK, (half + 1) * SBLK)
            nc.vector.tensor_add(dest[:, sl], t1, t2)

        def emit_v_unit(sc):
            if sc in done_v:
                return
            done_v.add(sc)
            cb, off = sc // 4, (sc % 4) * P
            ps = psB.tile([P, OV], f32, tag="acc")
            for d in range(dch):
                nc.tensor.matmul(
                    ps, xT[d][cb][:, off:off + P], wqv[d],
                    start=(d == 0), stop=(d == dch - 1))
            psv = ps.rearrange("p (a two d) -> p a two d", two=2, d=DK)
            va = const.tile([P, NPAIR, 65], bf16, tag=f"vA{sc}")
            vb = const.tile([P, NPAIR, P], bf16, tag=f"vB{sc}")
            nc.vector.tensor_copy(va[:, :, 0:DK], psv[:, :, 0, :])
            nc.vector.memset(va[:, :, DK:DK + 1], 1.0)
            nc.vector.memset(vb[:, :, 0:32], 0.0)
            nc.vector.memset(vb[:, :, 32:33], 1.0)
            nc.vector.memset(vb[:, :, 33:DK], 0.0)
            nc.vector.tensor_copy(vb[:, :, DK:2 * DK], psv[:, :, 1, :])
            for pr in range(NPAIR):
                vA[sc][pr] = va[:, pr, :]
                vB[sc][pr] = vb[:, pr, :]

        def qt_sl(pr, qb, lo, hi, rows):
            t = qt[pr][qb // 2]
            off = (qb % 2) * SBLK
            return t[rows[0]:rows[1]][:, off + lo:off + hi]

        def kt_sl(pr, kc, r0, r1):
            t = kt[pr][kc // 8]
            off = (kc % 8) * P
            return t[r0:r1, off:off + P]

        # ---- attention ------------------------------------------------------
        at = [[None] * NPAIR for _ in range(nsb)]

        def emit_norm(qb, pr, accA, accB):
            rtb = npool.tile([P, SBLK], bf16, tag="recipb")
            with nc.allow_low_precision(reason="bf16 softmax denom"):
                nc.vector.tensor_copy(
                    rtb[DK:DK + 1, :], accA[DK:DK + 1, :])
                nc.vector.tensor_copy(rtb[32:33, :], accB[32:33, :])
            rbp = psS.tile([P, W2], f32, tag="mm")
            nc.tensor.matmul(
                rbp[0:DK, 0:SBLK], ones_sb[DK:DK + 1, :],
                rtb[DK:DK + 1, :],
                start=True, stop=True, tile_position=(64, 0))
            nc.tensor.matmul(
                rbp[DK:P, 0:SBLK], ones_sb[32:33, :], rtb[32:33, :],
                start=True, stop=True, tile_position=(32, 64))
            rbs = npool.tile([P, SBLK], f32, tag="rbcast")
            nc.vector.reciprocal_approx_fast(rbs, rbp[:, 0:SBLK])
            atile = atpool.tile([P, SBLK], bf16, tag=f"at{pr}")
            nc.vector.tensor_tensor(
                atile[0:DK, :], accA[0:DK, :], rbs[0:DK, :], op=MUL)
            nc.vector.tensor_tensor(
                atile[DK:P, :], accB[DK:P, :], rbs[DK:P, :], op=MUL)
            at[qb][pr] = atile

        def emit_outproj(qb, qc):
            po = psS.tile([P, W2], f32, tag="mm")
            for nb in range(D // SBLK):
                for pr2 in range(NPAIR):
                    nc.tensor.matmul(
                        po[:, nb * SBLK:(nb + 1) * SBLK],
                        at[qb][pr2][:, qc * P:(qc + 1) * P],
                        wo[pr2][:, nb * SBLK:(nb + 1) * SBLK],
                        start=(pr2 == 0), stop=(pr2 == NPAIR - 1))
            osb = opool.tile([P, D], f32, tag="osb")
            nc.vector.tensor_copy(osb, po)
            q_glob = qb * SBLK + qc * P
            nc.sync.dma_start(out=out_d[q_glob:q_glob + P, :], in_=osb)

        # ordered projection worklist (demand order); pumped one unit at a
        # time inside the chunk loops so the exp stream on ACT never drains
        all_units = []
        seen = set()
        for qb in range(nsb):
            for pr in range(NPAIR):
                for sc in range(4 * qb, 4 * qb + 4):
                    if ("v", sc) not in seen:
                        seen.add(("v", sc))
                        all_units.append(("v", sc))
                for key in ([(True, pr, qb // 2, qb % 2)]
                            + [(False, pr, kcb // 2, kcb % 2)
                               for kcb in range(qb + 1)]):
                    if ("qk", key) not in seen:
                        seen.add(("qk", key))
                        all_units.append(("qk", key))
        unit_ix = [0]

        def pump_one():
            while unit_ix[0] < len(all_units):
                kind, args = all_units[unit_ix[0]]
                unit_ix[0] += 1
                if kind == "v":
                    if args not in done_v:
                        emit_v_unit(args)
                        return
                else:
                    if args not in done_qk:
                        emit_qk_unit(*args)
                        return

        for qb in range(nsb):
            pend = None
            for pr in range(NPAIR):
                # JIT projection for this (qb, pr) -- the PE work here fills
                # the pair-boundary while the previous pair's exp drains.
                for sc in range(4 * qb, 4 * qb + 4):
                    emit_v_unit(sc)
                emit_qk_unit(True, pr, qb // 2, qb % 2)
                # kt unit for kcb == qb feeds only the diagonal chunks
                # (kc >= 4*qb); for qb >= 1 it is emitted mid-loop instead
                for kcb in range(qb if qb >= 1 else qb + 1):
                    emit_qk_unit(False, pr, kcb // 2, kcb % 2)
                if pend is not None:
                    emit_norm(*pend)
                    pend = None
                accA = psB.tile([P, SBLK], f32, tag="acc")
                accB = psB.tile([P, SBLK], f32, tag="acc")
                nkc = 4 * qb + 4
                prev = None
                for kc in range(nkc):
                    diag_o = kc - 4 * qb
                    q0 = max(diag_o, 0) * P
                    sp = psS.tile([P, W2], f32, tag="mm")
                    nc.tensor.matmul(
                        sp[:, q0:SBLK],
                        kt_sl(pr, kc, 0, DK),
                        qt_sl(pr, qb, q0, SBLK, (0, DK)),
                        start=True, stop=True, tile_position=(0, 0))
                    nc.tensor.matmul(
                        sp[:, SBLK + q0:W2],
                        kt_sl(pr, kc, DK, P),
                        qt_sl(pr, qb, q0, SBLK, (DK, P)),
                        start=True, stop=True, tile_position=(64, 0))
                    pp = ppool.tile([P, W2], bf16, tag="p")
                    nc.scalar.activation(
                        pp.rearrange("p (h q) -> p h q", h=2)[:, :, q0:SBLK],
                        sp.rearrange("p (h q) -> p h q", h=2)[:, :, q0:SBLK],
                        EXP)
                    if diag_o >= 0:
                        sl = pp.rearrange(
                            "p (h q) -> p h q", h=2)[:, :, q0:q0 + P]
                        nc.vector.tensor_tensor(sl, sl, tri3, op=MUL)
                    if prev is not None:
                        pkc, ppp, pq0 = prev
                        nc.tensor.matmul(
                            accA[0:65, pq0:SBLK], vA[pkc][pr],
                            ppp[:, pq0:SBLK],
                            start=(pkc == 0), stop=False)
                        nc.tensor.matmul(
                            accB[0:P, pq0:SBLK], vB[pkc][pr],
                            ppp[:, SBLK + pq0:W2],
                            start=(pkc == 0), stop=False)
                    if qb >= 1 and kc == 2:
                        emit_qk_unit(False, pr, qb // 2, qb % 2)
                    prev = (kc, pp, q0)
                pkc, ppp, pq0 = prev
                nc.tensor.matmul(
                    accA[0:65, pq0:SBLK], vA[pkc][pr], ppp[:, pq0:SBLK],
                    start=False, stop=True)
                nc.tensor.matmul(
                    accB[0:P, pq0:SBLK], vB[pkc][pr],
                    ppp[:, SBLK + pq0:W2],
                    start=False, stop=True)
                pend = (qb, pr, accA, accB)
            emit_norm(*pend)
            for qc in range(SBLK // P):
                emit_outproj(qb, qc)

    nc.finalize()
    return nc


def _host_prep(x, Wqkv, Wout, s=S):
    perm = _rope_perm()
    cosq, sinq, cosk, sink, swapm, tri2 = _host_tables(s)
    in_maps = []
    for c in range(NCORES):
        b, hh = c // 2, c % 2
        rows = []
        for sect in range(3):
            base = sect * D + hh * OV
            for h in range(HLOC):
                r = base + h * DK + (perm if sect < 2 else np.arange(DK))
                rows.append(r)
        idx = np.concatenate(rows)
        wslice = Wqkv[idx, :]
        in_maps.append({
            "xT": np.ascontiguousarray(x[b].T).astype(BF16),
            "wqkvT": np.ascontiguousarray(wslice.T).astype(BF16),
            "woutT": np.ascontiguousarray(
                Wout[:, hh * OV:(hh + 1) * OV].T).astype(BF16),
            "cosq": cosq, "sinq": sinq, "cosk": cosk, "sink": sink,
            "swapm": swapm, "tri2": tri2,
        })
    return in_maps


def kernel(x, Wqkv, Wout):
    from concourse.bass_utils import run_bass_kernel_spmd

    x = np.asarray(x, dtype=np.float32)
    Wqkv = np.asarray(Wqkv, dtype=np.float32)
    Wout = np.asarray(Wout, dtype=np.float32)

    nc = _build_nc(S)
    in_maps = _host_prep(x, Wqkv, Wout, S)
    res = run_bass_kernel_spmd(nc, in_maps, core_ids=list(range(NCORES)))
    outs = res.results
    out = np.empty((B, S, D), np.float32)
    for b in range(B):
        out[b] = outs[2 * b]["out"] + outs[2 * b + 1]["out"]
    return out


# revision 34
# speedup vs baseline: 1.1108x; 1.1108x over previous
"""Causal multi-head self-attention on 8 TRN2 NeuronCores.

Sharding: core c handles batch b = c//2 and head-half hh = c%2 (8 of 16
heads).  Each core computes qkv projection for its heads, RoPE, causal
attention, and a PARTIAL output projection (its heads' contribution to
Wout @ attn).  The host sums the two half-head partials per batch.
No collectives.

Attention uses TRANSPOSED scores sT[k, q] (no on-chip transposes); two
heads are row-packed into the PE array per score matmul; exp runs as one
ACT op per chunk-pair; attn@V uses a ones-column for the softmax sums
(head A: lhsT=[v|1] M=65, sums on partition 64; head B:
lhsT=[0(32)|1|0(31)|v] M=128, sums on partition 32, output on partitions
64-127 -- keeps every vector op lane-aligned).  Normalization: broadcast
the sums row with a K=1 ones-matmul, fast approximate reciprocal on the
broadcast, one tensor_mul per head fused with the psum->sbuf cast.

The qkv projection + RoPE is emitted JUST-IN-TIME, woven into the
attention pair boundaries, so the (ACT-exp-bound) attention phase hides
the projection's PE work.  Projection psum cycles through the attn-
accumulator pool's slots serially (ps half -> release -> swap psum).
"""

import numpy as np
import ml_dtypes

BF16 = ml_dtypes.bfloat16

B, S, D = 4, 2048, 1024
H, DK = 16, 64
THETA = 10000.0
NCORES = 8
HLOC = H // 2
NPAIR = HLOC // 2
P = 128
SBLK = 512
OV = HLOC * DK
WCOLS = 3 * OV


def _rope_perm():
    return np.concatenate([np.arange(0, DK, 2), np.arange(1, DK, 2)])


def _host_tables(s):
    half = DK // 2
    inv_freq = THETA ** (-np.arange(0, DK, 2, dtype=np.float64) / DK)
    pos = np.arange(s, dtype=np.float64)
    ang = pos[None, :] * inv_freq[:, None]
    c, sn = np.cos(ang), np.sin(ang)
    cos_t = np.empty((P, s), np.float32)
    sin_t = np.empty((P, s), np.float32)
    for hrow in range(2):
        o = hrow * DK
        cos_t[o:o + half] = c
        cos_t[o + half:o + DK] = c
        sin_t[o:o + half] = -sn
        sin_t[o + half:o + DK] = sn
    scale = 1.0 / np.sqrt(DK)
    cosq = (cos_t * scale).astype(BF16)
    sinq = (sin_t * scale).astype(BF16)
    cosk = cos_t.astype(BF16)
    sink = sin_t.astype(BF16)

    swap = np.zeros((P, P), np.float32)
    for hrow in range(2):
        o = hrow * DK
        for i in range(half):
            swap[o + i, o + half + i] = 1.0
            swap[o + half + i, o + i] = 1.0
    swapm = swap.astype(BF16)

    tri = (np.arange(P)[:, None] <= np.arange(P)[None, :]).astype(np.float32)
    tri2 = np.concatenate([tri, tri], axis=1).astype(BF16)
    return cosq, sinq, cosk, sink, swapm, tri2


def _build_nc(s=S):
    import concourse.bass as bass  # noqa: F401
    import concourse.mybir as mybir
    import concourse.tile as tile
    from concourse import bacc
    from contextlib import ExitStack

    f32 = mybir.dt.float32
    bf16 = mybir.dt.bfloat16
    EXP = mybir.ActivationFunctionType.Exp
    MUL = mybir.AluOpType.mult

    nsb = s // SBLK
    nqc = s // P
    dch = D // P
    assert nsb % 2 == 0

    nc = bacc.Bacc(None, target_bir_lowering=False)
    xT_d = nc.dram_tensor("xT", [D, s], bf16, kind="ExternalInput")
    wq_d = nc.dram_tensor("wqkvT", [D, WCOLS], bf16, kind="ExternalInput")
    wo_d = nc.dram_tensor("woutT", [OV, D], bf16, kind="ExternalInput")
    cosq_d = nc.dram_tensor("cosq", [P, s], bf16, kind="ExternalInput")
    sinq_d = nc.dram_tensor("sinq", [P, s], bf16, kind="ExternalInput")
    cosk_d = nc.dram_tensor("cosk", [P, s], bf16, kind="ExternalInput")
    sink_d = nc.dram_tensor("sink", [P, s], bf16, kind="ExternalInput")
    swap_d = nc.dram_tensor("swapm", [P, P], bf16, kind="ExternalInput")
    tri_d = nc.dram_tensor("tri2", [P, 2 * P], bf16, kind="ExternalInput")
    out_d = nc.dram_tensor("out", [s, D], f32, kind="ExternalOutput")

    W2 = 2 * SBLK

    with tile.TileContext(nc) as tc, ExitStack() as ctx:
        const = ctx.enter_context(tc.tile_pool(name="const", bufs=1))
        psS = ctx.enter_context(
            tc.tile_pool(name="psS", bufs=2, space="PSUM"))
        psB = ctx.enter_context(
            tc.tile_pool(name="psB", bufs=4, space="PSUM"))
        rpool = ctx.enter_context(tc.tile_pool(name="rope", bufs=2))
        ppool = ctx.enter_context(tc.tile_pool(name="probs", bufs=6))
        npool = ctx.enter_context(tc.tile_pool(name="norm", bufs=2))
        opool = ctx.enter_context(tc.tile_pool(name="outsb", bufs=2))
        atpool = ctx.enter_context(tc.tile_pool(name="attnT", bufs=2))

        # ---- staged constant loads -----------------------------------------
        # order matters: v weights + first x column block land first so the
        # first v-projection units can start ~immediately.
        NCB = s // SBLK                     # x column blocks
        xT = [[None] * NCB for _ in range(dch)]
        wqv = []
        wqk = []
        for d in range(dch):
            t = const.tile([P, OV], bf16, tag=f"wqv{d}")
            nc.sync.dma_start(out=t, in_=wq_d[d * P:(d + 1) * P, 2 * OV:])
            wqv.append(t)
        for d in range(dch):
            t = const.tile([P, SBLK], bf16, tag=f"xT{d}_0")
            nc.sync.dma_start(out=t, in_=xT_d[d * P:(d + 1) * P, 0:SBLK])
            xT[d][0] = t
        for d in range(dch):
            t = const.tile([P, 2 * OV], bf16, tag=f"wqk{d}")
            nc.sync.dma_start(out=t, in_=wq_d[d * P:(d + 1) * P, 0:2 * OV])
            wqk.append(t)
        tabs = {}
        for nm, dram in (("cosq", cosq_d), ("sinq", sinq_d),
                         ("cosk", cosk_d), ("sink", sink_d)):
            t = const.tile([P, s], bf16, tag=nm)
            nc.sync.dma_start(out=t, in_=dram[:, :])
            tabs[nm] = t
        swap_sb = const.tile([P, P], bf16, tag="swapm")
        nc.sync.dma_start(out=swap_sb, in_=swap_d[:, :])
        tri_sb = const.tile([P, 2 * P], bf16, tag="tri2")
        nc.sync.dma_start(out=tri_sb, in_=tri_d[:, :])
        tri3 = tri_sb.rearrange("p (h q) -> p h q", h=2)
        ones_sb = const.tile([P, DK], bf16, tag="ones")
        nc.vector.memset(ones_sb, 1.0)
        # HAM warm-up: keep the PE busy on scratch data while the input DMAs
        # stream in, so the real matmuls start at the full 2.4 GHz clock.
        warm = const.tile([P, SBLK], bf16, tag="warm")
        nc.vector.memset(warm, 1.0)
        wps = psB.tile([P, SBLK], f32, tag="acc")
        for _ in range(24):
            nc.tensor.matmul(wps, warm[:, 0:P], warm,
                             start=True, stop=True)
        for cb in range(1, NCB):
            for d in range(dch):
                t = const.tile([P, SBLK], bf16, tag=f"xT{d}_{cb}")
                nc.sync.dma_start(
                    out=t, in_=xT_d[d * P:(d + 1) * P,
                                    cb * SBLK:(cb + 1) * SBLK])
                xT[d][cb] = t
        wo = []
        for i in range(OV // P):
            t = const.tile([P, D], bf16, tag=f"wo{i}")
            nc.sync.dma_start(out=t, in_=wo_d[i * P:(i + 1) * P, :])
            wo.append(t)

        # ---- projection units (emitted JIT) --------------------------------
        qt = [[None] * (nsb // 2) for _ in range(NPAIR)]   # [128, 1024]
        kt = [[None] * (nsb // 2) for _ in range(NPAIR)]
        vA = [[None] * NPAIR for _ in range(nqc)]
        vB = [[None] * NPAIR for _ in range(nqc)]
        done_qk = set()
        done_v = set()

        def emit_qk_unit(is_q, pr, sbp, half):
            key = (is_q, pr, sbp, half)
            if key in done_qk:
                return
            done_qk.add(key)
            dests = qt if is_q else kt
            if dests[pr][sbp] is None:
                nm = ("qt" if is_q else "kt") + f"{pr}_{sbp}"
                dests[pr][sbp] = const.tile(
                    [P, W2], bf16, tag=nm, name=nm)
            dest = dests[pr][sbp]
            wcol = (pr if is_q else NPAIR + pr) * P
            ct = tabs["cosq"] if is_q else tabs["cosk"]
            st = tabs["sinq"] if is_q else tabs["sink"]
            cb = 2 * sbp + half
            ps = psB.tile([P, SBLK], f32, tag="acc")
            for d in range(dch):
                nc.tensor.matmul(
                    ps, wqk[d][:, wcol:wcol + P], xT[d][cb],
                    start=(d == 0), stop=(d == dch - 1))
            c0 = cb * SBLK
            y = rpool.tile([P, SBLK], bf16, tag="y")
            nc.scalar.copy(y, ps)
            sw = psB.tile([P, SBLK], f32, tag="acc")
            nc.tensor.matmul(sw, swap_sb, y, start=True, stop=True)
            t1 = rpool.tile([P, SBLK], bf16, tag="t1")
            nc.vector.tensor_mul(t1, y, ct[:, c0:c0 + SBLK])
            t2 = rpool.tile([P, SBLK], bf16, tag="t2")
            nc.vector.tensor_mul(t2, sw, st[:, c0:c0 + SBLK])
            sl = slice(half * SBLK, (half + 1) * SBLK)
            nc.vector.tensor_add(dest[:, sl], t1, t2)

        def emit_v_unit(sc):
            if sc in done_v:
                return
            done_v.add(sc)
            cb, off = sc // 4, (sc % 4) * P
            ps = psB.tile([P, OV], f32, tag="acc")
            for d in range(dch):
                nc.tensor.matmul(
                    ps, xT[d][cb][:, off:off + P], wqv[d],
                    start=(d == 0), stop=(d == dch - 1))
            psv = ps.rearrange("p (a two d) -> p a two d", two=2, d=DK)
            va = const.tile([P, NPAIR, 65], bf16, tag=f"vA{sc}")
            vb = const.tile([P, NPAIR, P], bf16, tag=f"vB{sc}")
            nc.vector.tensor_copy(va[:, :, 0:DK], psv[:, :, 0, :])
            nc.vector.memset(va[:, :, DK:DK + 1], 1.0)
            nc.vector.memset(vb[:, :, 0:32], 0.0)
            nc.vector.memset(vb[:, :, 32:33], 1.0)
            nc.vector.memset(vb[:, :, 33:DK], 0.0)
            nc.vector.tensor_copy(vb[:, :, DK:2 * DK], psv[:, :, 1, :])
            for pr in range(NPAIR):
                vA[sc][pr] = va[:, pr, :]
                vB[sc][pr] = vb[:, pr, :]

        def qt_sl(pr, qb, lo, hi, rows):
            t = qt[pr][qb // 2]
            off = (qb % 2) * SBLK
            return t[rows[0]:rows[1]][:, off + lo:off + hi]

        def kt_sl(pr, kc, r0, r1):
            t = kt[pr][kc // 8]
            off = (kc % 8) * P
            return t[r0:r1, off:off + P]

        # ---- attention ------------------------------------------------------
        at = [[None] * NPAIR for _ in range(nsb)]

        def emit_norm(qb, pr, accA, accB):
            rtb = npool.tile([P, SBLK], bf16, tag="recipb")
            with nc.allow_low_precision(reason="bf16 softmax denom"):
                nc.vector.tensor_copy(
                    rtb[DK:DK + 1, :], accA[DK:DK + 1, :])
                nc.vector.tensor_copy(rtb[32:33, :], accB[32:33, :])
            rbp = psS.tile([P, W2], f32, tag="mm")
            nc.tensor.matmul(
                rbp[0:DK, 0:SBLK], ones_sb[DK:DK + 1, :],
                rtb[DK:DK + 1, :],
                start=True, stop=True, tile_position=(64, 0))
            nc.tensor.matmul(
                rbp[DK:P, 0:SBLK], ones_sb[32:33, :], rtb[32:33, :],
                start=True, stop=True, tile_position=(32, 64))
            rbs = npool.tile([P, SBLK], f32, tag="rbcast")
            nc.vector.reciprocal_approx_fast(rbs, rbp[:, 0:SBLK])
            atile = atpool.tile([P, SBLK], bf16, tag=f"at{pr}")
            nc.vector.tensor_tensor(
                atile[0:DK, :], accA[0:DK, :], rbs[0:DK, :], op=MUL)
            nc.vector.tensor_tensor(
                atile[DK:P, :], accB[DK:P, :], rbs[DK:P, :], op=MUL)
            at[qb][pr] = atile

        def emit_outproj(qb, qc):
            po = psS.tile([P, W2], f32, tag="mm")
            for nb in range(D // SBLK):
                for pr2 in range(NPAIR):
                    nc.tensor.matmul(
                        po[:, nb * SBLK:(nb + 1) * SBLK],
                        at[qb][pr2][:, qc * P:(qc + 1) * P],
                        wo[pr2][:, nb * SBLK:(nb + 1) * SBLK],
                        start=(pr2 == 0), stop=(pr2 == NPAIR - 1))
            osb = opool.tile([P, D], f32, tag="osb")
            nc.vector.tensor_copy(osb, po)
            q_glob = qb * SBLK + qc * P
            nc.sync.dma_start(out=out_d[q_glob:q_glob + P, :], in_=osb)

        # ordered projection worklist (demand order); pumped one unit at a
        # time inside the chunk loops so the exp stream on ACT never drains
        all_units = []
        seen = set()
        for qb in range(nsb):
            for pr in range(NPAIR):
                for sc in range(4 * qb, 4 * qb + 4):
                    if ("v", sc) not in seen:
                        seen.add(("v", sc))
                        all_units.append(("v", sc))
                for key in ([(True, pr, qb // 2, qb % 2)]
                            + [(False, pr, kcb // 2, kcb % 2)
                               for kcb in range(qb + 1)]):
                    if ("qk", key) not in seen:
                        seen.add(("qk", key))
                        all_units.append(("qk", key))
        unit_ix = [0]

        def pump_one():
            while unit_ix[0] < len(all_units):
                kind, args = all_units[unit_ix[0]]
                unit_ix[0] += 1
                if kind == "v":
                    if args not in done_v:
                        emit_v_unit(args)
                        return
                else:
                    if args not in done_qk:
                        emit_qk_unit(*args)
                        return

        for qb in range(nsb):
            pend = None
            for pr in range(NPAIR):
                # JIT projection for this (qb, pr) -- the PE work here fills
                # the pair-boundary while the previous pair's exp drains.
                for sc in range(4 * qb, 4 * qb + 4):
                    emit_v_unit(sc)
                emit_qk_unit(True, pr, qb // 2, qb % 2)
                for kcb in range(qb + 1):
                    emit_qk_unit(False, pr, kcb // 2, kcb % 2)
                if pend is not None:
                    emit_norm(*pend)
                    pend = None
                accA = psB.tile([P, SBLK], f32, tag="acc")
                accB = psB.tile([P, SBLK], f32, tag="acc")
                nkc = 4 * qb + 4
                prev = None
                for kc in range(nkc):
                    diag_o = kc - 4 * qb
                    q0 = max(diag_o, 0) * P
                    sp = psS.tile([P, W2], f32, tag="mm")
                    nc.tensor.matmul(
                        sp[:, q0:SBLK],
                        kt_sl(pr, kc, 0, DK),
                        qt_sl(pr, qb, q0, SBLK, (0, DK)),
                        start=True, stop=True, tile_position=(0, 0))
                    nc.tensor.matmul(
                        sp[:, SBLK + q0:W2],
                        kt_sl(pr, kc, DK, P),
                        qt_sl(pr, qb, q0, SBLK, (DK, P)),
                        start=True, stop=True, tile_position=(64, 0))
                    pp = ppool.tile([P, W2], bf16, tag="p")
                    nc.scalar.activation(
                        pp.rearrange("p (h q) -> p h q", h=2)[:, :, q0:SBLK],
                        sp.rearrange("p (h q) -> p h q", h=2)[:, :, q0:SBLK],
                        EXP)
                    if diag_o >= 0:
                        sl = pp.rearrange(
                            "p (h q) -> p h q", h=2)[:, :, q0:q0 + P]
                        nc.vector.tensor_tensor(sl, sl, tri3, op=MUL)
                    if prev is not None:
                        pkc, ppp, pq0 = prev
                        nc.tensor.matmul(
                            accA[0:65, pq0:SBLK], vA[pkc][pr],
                            ppp[:, pq0:SBLK],
                            start=(pkc == 0), stop=False)
                        nc.tensor.matmul(
                            accB[0:P, pq0:SBLK], vB[pkc][pr],
                            ppp[:, SBLK + pq0:W2],
                            start=(pkc == 0), stop=False)
                    prev = (kc, pp, q0)
                pkc, ppp, pq0 = prev
                nc.tensor.matmul(
                    accA[0:65, pq0:SBLK], vA[pkc][pr], ppp[:, pq0:SBLK],
                    start=False, stop=True)
                nc.tensor.matmul(
                    accB[0:P, pq0:SBLK], vB[pkc][pr],
                    ppp[:, SBLK + pq0:W2],
                    start=False, stop=True)
                pend = (qb, pr, accA, accB)
            emit_norm(*pend)
            for qc in range(SBLK // P):
                emit_outproj(qb, qc)

    nc.finalize()
    return nc


def _host_prep(x, Wqkv, Wout, s=S):
    perm = _rope_perm()
    cosq, sinq, cosk, sink, swapm, tri2 = _host_tables(s)
    in_maps = []
    for c in range(NCORES):
        b, hh = c // 2, c % 2
        rows = []
        for sect in range(3):
            base = sect * D + hh * OV
            for h in range(HLOC):
                r = base + h * DK + (perm if sect < 2 else np.arange(DK))
                rows.append(r)
        idx = np.concatenate(rows)
        wslice = Wqkv[idx, :]
        in_maps.append({
            "xT": np.ascontiguousarray(x[b].T).astype(BF16),
            "wqkvT": np.ascontiguousarray(wslice.T).astype(BF16),
            "woutT": np.ascontiguousarray(
                Wout[:, hh * OV:(hh + 1) * OV].T).astype(BF16),
            "cosq": cosq, "sinq": sinq, "cosk": cosk, "sink": sink,
            "swapm": swapm, "tri2": tri2,
        })
    return in_maps


def kernel(x, Wqkv, Wout):
    from concourse.bass_utils import run_bass_kernel_spmd

    x = np.asarray(x, dtype=np.float32)
    Wqkv = np.asarray(Wqkv, dtype=np.float32)
    Wout = np.asarray(Wout, dtype=np.float32)

    nc = _build_nc(S)
    in_maps = _host_prep(x, Wqkv, Wout, S)
    res = run_bass_kernel_spmd(nc, in_maps, core_ids=list(range(NCORES)))
    outs = res.results
    out = np.empty((B, S, D), np.float32)
    for b in range(B):
        out[b] = outs[2 * b]["out"] + outs[2 * b + 1]["out"]
    return out
